# revision 38
# baseline (speedup 1.0000x reference)
import sys
if '/opt/trn_rl_repo' not in sys.path:
    sys.path.insert(0, '/opt/trn_rl_repo')
import numpy as np
import ml_dtypes

import concourse.bass as bass
import concourse.bacc as bacc
import concourse.tile as tile
from concourse import mybir
from concourse.bass_utils import run_bass_kernel_spmd
from concourse.masks import make_identity

F32 = mybir.dt.float32
BF = mybir.dt.bfloat16
E4 = mybir.dt.float8e4
AF = mybir.ActivationFunctionType
MUL = mybir.AluOpType.mult
ADD = mybir.AluOpType.add
SUB = mybir.AluOpType.subtract
DR = mybir.MatmulPerfMode.DoubleRow
P = 128
NPE4 = ml_dtypes.float8_e4m3
SW = 64.0     # fp8 weight pre-scale (host); descale folded into exp/fc/cqa
SXB = 8.0     # fp8 cqa xblock pre-scale (device)
D, H, DK, DV, NL = 768, 8, 64, 64, 2
B, LC, LQ, LK = 8, 512, 160, 512
DC = D // P      # 6 chunks of the 768 dim
CC = LC // P     # 4 chunks of the 512 token dim
QCH = [(0, 128), (128, 32)]   # (offset, size) chunks of LQ=160
SCALE = 0.125    # log_512(512)/sqrt(64)
EPS = 1e-6
NPBF = ml_dtypes.bfloat16

USE_SPART = True
USE_APPROX_RECIP = True
USE_GPS_BCAST = True
_CACHE = {}


def _build():
    nc = bacc.Bacc()
    dt = {}

    def din(name, shape, dtype=BF):
        dt[name] = nc.dram_tensor(name, list(shape), dtype, kind="ExternalInput")
        return dt[name]

    # all big tensors pre-tiled on host to [128, chunks*width] (partition-contiguous)
    din('S_nat', (P, CC * D)); din('S_T', (P, DC * LC))
    din('Q_nat', (P, 2 * D)); din('Q_T', (P, DC * LQ))
    din('E_nat', (P, 2 * D)); din('E_T', (P, DC * LQ))
    din('KE_T', (P, DC * LK)); din('KE8', (P, DC * LK), E4)
    din('vecs', (P, DC * 4), F32)    # cols: w4C, w4Q, w4mlu, cqa_b
    din('cqa_WTs', (P, DC * D))
    din('cqa_WT8', (P, 3 * DC * D), E4)
    for l in range(NL):
        din(f'sWq{l}', (P, DC * H * DK), E4); din(f'sWk{l}', (P, DC * H * DK), E4)
        din(f'sWv{l}', (P, DC * H * DV), E4); din(f'sWfc{l}', (P, 4 * D), E4)
        din(f'cWq{l}', (P, DC * H * DK), E4); din(f'cWk{l}', (P, 2 * DC * H * DK), E4)
        din(f'cWv{l}', (P, 2 * DC * H * DV), E4); din(f'cWfc{l}', (P, 4 * D), E4)
        din(f'ln{l}', (P, DC * 4), F32)   # cols: n1g, n1b, n2g, n2b
    out_t = nc.dram_tensor('out_t', [3 * D, LC], BF, kind="ExternalOutput")

    with tile.TileContext(nc) as tc:
        _emit(nc, tc, dt, out_t)
    nc.compile()
    return nc


def _emit(nc, tc, dt, out_t):
    from contextlib import ExitStack
    ctx = ExitStack()
    const = ctx.enter_context(tc.tile_pool(name="const", bufs=1))
    persist = ctx.enter_context(tc.tile_pool(name="persist", bufs=1))

    ident = const.tile([P, P], BF)
    make_identity(nc, ident)
    ones_row = const.tile([1, P], BF)
    nc.gpsimd.memset(ones_row, 1.0)
    ones_col = const.tile([P, 1], BF)
    nc.gpsimd.memset(ones_col, 1.0)
    eps_t = const.tile([1, 1], F32)
    nc.gpsimd.memset(eps_t, EPS)


    # ---- weight pool for layer 0 (tiles alloc'd early for LIFO order;
    # DMAs emitted after the input DMAs) ----
    def alloc_wl(l, wl, names):
        w = {}
        for nm in names:
            if nm in ('sWfc', 'cWfc'):
                w[nm] = wl.tile([P, 4, D], E4, name=f"{nm}{l}")
            elif nm == 'sWv':
                w[nm] = wl.tile([P, DC, H * DV], E4, name=f"{nm}{l}")
            else:
                w[nm] = wl.tile([P, DC, H * DK], E4, name=f"{nm}{l}")
        return w

    def dma_wl(l, w, names=None):
        for nm in (names if names is not None else w):
            nc.sync.dma_start(out=w[nm], in_=dt[f'{nm}{l}'][:, :])

    wl0pool = ctx.enter_context(tc.tile_pool(name="wl0", bufs=1))
    wls = [alloc_wl(0, wl0pool, ('sWq', 'sWk', 'sWv', 'sWfc')), None]
    ps = ctx.enter_context(tc.tile_pool(name="ps", bufs=1, space="PSUM"))

    # ---- input DMAs (batched; earliest-needed first) ----
    s2q = tc.alloc_tile_pool(name="s2q", bufs=1)
    cqaw = tc.alloc_tile_pool(name="cqaw", bufs=1)

    vecs = const.tile([P, DC, 4], F32)
    nc.sync.dma_start(out=vecs, in_=dt['vecs'][:, :])
    vecs_bf = const.tile([P, DC, 4], BF)
    nc.vector.tensor_copy(vecs_bf, vecs)

    ST3 = s2q.tile([P, DC, LC], BF, name="ST3")
    nc.sync.dma_start(out=ST3[:, 0:3, :], in_=dt['S_T'][:, 0:3 * LC])
    nc.sync.dma_start(out=ST3[:, 3:6, :], in_=dt['S_T'][:, 3 * LC:])
    S_T = [ST3[:, d, :] for d in range(DC)]

    qe_in = {}
    for tag, QN, QT in (('q', dt['Q_nat'], dt['Q_T']), ('e', dt['E_nat'], dt['E_T'])):
        qt3 = s2q.tile([P, DC, LQ], BF, name=f"Qt3{tag}")
        nc.sync.dma_start(out=qt3[:, 0:3, :], in_=QT[:, 0:3 * LQ])
        nc.sync.dma_start(out=qt3[:, 3:6, :], in_=QT[:, 3 * LQ:])
        qn3 = s2q.tile([P, 2, D], BF, name=f"Qn3{tag}")
        nc.sync.dma_start(out=qn3, in_=QN[:, :])
        Qn = [qn3[:, 0, :], qn3[:, 1, :]]
        qe_in[tag] = ([qt3[:, d, :] for d in range(DC)], Qn)

    KE3 = persist.tile([P, DC, LK], BF, name="KE3")
    nc.sync.dma_start(out=KE3, in_=dt['KE_T'][:, :])
    ke_T = [KE3[:, d, :] for d in range(DC)]
    KE83 = persist.tile([P, DC, LK], E4, name="KE83")
    nc.sync.dma_start(out=KE83, in_=dt['KE8'][:, :])

    dma_wl(0, wls[0], ('sWq', 'sWk'))

    SN3 = s2q.tile([P, CC, D], BF, name="SN3")
    nc.sync.dma_start(out=SN3, in_=dt['S_nat'][:, :])
    S_nat = [SN3[:, c, :] for c in range(CC)]

    dma_wl(0, wls[0], ('sWv',))

    CQ3s = cqaw.tile([P, DC, D], BF, name="CQ3s")
    nc.sync.dma_start(out=CQ3s, in_=dt['cqa_WTs'][:, :])
    cqa_WT = [CQ3s[:, k, :] for k in range(DC)]
    CQ8 = cqaw.tile([P, 3 * DC, D], E4, name="CQ8")
    nc.sync.dma_start(out=CQ8, in_=dt['cqa_WT8'][:, :])

    dma_wl(0, wls[0], ('sWfc',))

    lnv = []
    for l in range(NL):
        t = const.tile([P, DC, 4], F32, name=f"lnv{l}")
        nc.sync.dma_start(out=t, in_=dt[f'ln{l}'][:, :])
        lnv.append(t)

    # ---- S-side shared prep ----
    cm3 = s2q.tile([P, DC, LC], BF, name="cm3")
    cm_T = [cm3[:, d, :] for d in range(DC)]
    for d in range(DC):
        nc.vector.tensor_scalar_mul(cm_T[d], S_T[d], vecs[:, d, 2:3])
    ps0 = ps.tile([1, LC], F32, tag="b", bufs=3)
    for d in range(DC):
        nc.tensor.matmul(ps0, vecs_bf[:, d, 0:1], S_T[d], start=(d == 0), stop=(d == DC - 1))
    s0_row = s2q.tile([1, LC], BF)
    nc.vector.tensor_copy(s0_row, ps0)
    # att held in SBUF (fp8) for phase 2 cross-attn kv
    att8 = persist.tile([P, 2 * DC, LC], E4, name="att8")

    # ---- s2q in stages; q/e interleaved ----
    pools = {}
    st = {}

    def stageA(tag):
        po = tc.alloc_tile_pool(name=f"s2qt_{tag}", bufs=1)
        pools[tag] = po
        Qt, Qn = qe_in[tag]
        s1 = []
        for qi, (qo, qs) in enumerate(QCH):
            pq = ps.tile([P, 1], F32, tag="b", bufs=3)
            for d in range(DC):
                nc.tensor.matmul(pq[:qs], Qt[d][:, qo:qo + qs], vecs_bf[:, d, 1:2],
                                 start=(d == 0), stop=(d == DC - 1))
            t = po.tile([P, 1], F32, name=f"s1{tag}{qi}")
            nc.vector.tensor_copy(t[:qs], pq[:qs])
            s1.append(t)
        e_t, etn = [], []
        for qi, (qo, qs) in enumerate(QCH):
            psc_t = ps.tile([P, LC], F32, tag="a", bufs=3)
            for d in range(DC):
                nc.tensor.matmul(psc_t[:qs], Qt[d][:, qo:qo + qs], cm_T[d],
                                 start=(d == 0), stop=False)
            nc.tensor.matmul(psc_t[:qs], ones_row[:1, :qs], s0_row,
                             start=False, stop=True)
            et = po.tile([P, LC], BF, name=f"et{tag}{qi}")
            stt = po.tile([P, 1], F32, name=f"st{tag}{qi}")
            nc.scalar.activation(et[:qs], psc_t[:qs], AF.Exp, bias=s1[qi][:qs],
                                 scale=1.0, accum_out=stt[:qs])
            rt = po.tile([P, 1], F32, name=f"rt{tag}{qi}")
            nc.vector.reciprocal_approx_fast(out=rt[:qs], in_=stt[:qs])
            en = po.tile([P, LC], BF, name=f"etn{tag}{qi}")
            nc.vector.tensor_scalar_mul(en[:qs], et[:qs], rt[:qs])
            e_t.append(et); etn.append(en)
        psr = ps.tile([1, LC], F32, tag="b", bufs=3)
        for qi, (qo, qs) in enumerate(QCH):
            nc.tensor.matmul(psr, ones_col[:qs, :1], e_t[qi][:qs],
                             start=(qi == 0), stop=(qi == 1))
        rc_row = po.tile([1, LC], F32, name=f"rc{tag}")
        nc.scalar.copy(rc_row, psr)
        nc.vector.reciprocal_approx_fast(out=rc_row, in_=rc_row)
        st[tag] = dict(e_t=e_t, etn=etn, rc_row=rc_row)

    def stageB(tag):
        po = pools[tag]
        s = st[tag]
        rcb = po.tile([1, LC], BF, name=f"rcb{tag}")
        nc.vector.tensor_copy(rcb, s['rc_row'])
        pbs = ps.tile([P, LC], F32, tag="b", bufs=3)
        nc.tensor.matmul(pbs, ones_row, rcb, start=True, stop=True)
        P_T = []
        for qi, (qo, qs) in enumerate(QCH):
            pt = po.tile([P, LC], BF, name=f"PT{tag}{qi}")
            nc.vector.tensor_tensor(pt[:qs], s['e_t'][qi][:qs], pbs[:qs], op=MUL)
            P_T.append(pt)
        etn_T = [po.tile([P, LQ], BF, name=f"etnT{tag}{c}") for c in range(CC)]
        for c in range(CC):
            for qi, (qo, qs) in enumerate(QCH):
                pt = ps.tile([P, P], BF, tag="b", bufs=3)
                nc.tensor.transpose(pt[:, :qs], s['etn'][qi][:qs, c * P:(c + 1) * P],
                                    ident[:qs, :qs])
                nc.vector.tensor_copy(etn_T[c][:, qo:qo + qs], pt[:, :qs])
        tmp = []
        for qi, (qo, qs) in enumerate(QCH):
            t = po.tile([P, D], BF, name=f"tmp{tag}{qi}")
            for n in range(2):
                pm = ps.tile([P, 384], F32, tag="a", bufs=3)
                for c in range(CC):
                    nc.tensor.matmul(pm[:qs], etn_T[c][:, qo:qo + qs],
                                     S_nat[c][:, n * 384:(n + 1) * 384],
                                     start=(c == 0), stop=(c == CC - 1))
                nc.vector.tensor_copy(t[:qs, n * 384:(n + 1) * 384], pm[:qs])
            tmp.append(t)
        s['P_T'] = P_T; s['tmp'] = tmp

    def stageC(tag, row0):
        po = pools[tag]
        s = st[tag]
        Qt, Qn = qe_in[tag]
        P_T, tmp = s['P_T'], s['tmp']
        xb8 = po.tile([P, 3 * DC, LC], E4, name=f"xb8{tag}")
        for d in range(DC):
            pc = ps.tile([P, LC], F32, tag="a", bufs=3)
            for qi, (qo, qs) in enumerate(QCH):
                nc.tensor.matmul(pc, Qn[qi][:qs, d * P:(d + 1) * P], P_T[qi][:qs],
                                 start=(qi == 0), stop=(qi == 1))
            nc.vector.tensor_scalar_mul(xb8[:, d, :], pc, SXB)
            nc.vector.scalar_tensor_tensor(xb8[:, DC + d, :], pc, SXB, S_T[d],
                                           op0=MUL, op1=MUL)
            pq2 = ps.tile([P, LC], F32, tag="a", bufs=3)
            for qi, (qo, qs) in enumerate(QCH):
                nc.tensor.matmul(pq2, tmp[qi][:qs, d * P:(d + 1) * P], P_T[qi][:qs],
                                 start=(qi == 0), stop=(qi == 1))
            nc.vector.scalar_tensor_tensor(xb8[:, 2 * DC + d, :], pq2, SXB,
                                           S_T[d], op0=MUL, op1=MUL)
        for mc in range(DC):
            pco = ps.tile([P, LC], F32, tag="a", bufs=3)
            nc.tensor.matmul(pco, ident, Spart[mc], start=True, stop=False,
                             skip_group_check=True)
            for j in range(3 * DC // 2):
                nc.tensor.matmul(pco, CQ8[:, 2 * j:2 * j + 2, mc * P:(mc + 1) * P],
                                 xb8[:, 2 * j:2 * j + 2, :], start=False,
                                 stop=(j == 3 * DC // 2 - 1), perf_mode=DR,
                                 skip_group_check=True)
            ob = po.tile([P, LC], BF, name=f"ob{tag}{mc}", tag="attb", bufs=2)
            nc.scalar.activation(ob, pco, AF.Identity,
                                 bias=vecs[:, mc, 3:4], scale=1.0 / (SW * SXB))
            nc.sync.dma_start(out=out_t[(row0 + mc) * P:(row0 + mc + 1) * P, :],
                              in_=ob)
            nc.vector.tensor_copy(att8[:, row0 + mc, :], ob)

    def proj_early(wt3, rhs3, nk, nm):
        outs = [persist.tile([P, LK], BF, name=f"pe_{nm}{m}") for m in range(4)]
        for mg in range(0, 4, 2):
            pss = [ps.tile([P, LK], F32, name=f"pe_ps{nm}{mg+i}", tag="b", bufs=3)
                   for i in range(2)]
            for kp in range(nk // 2):
                for i in range(2):
                    m = mg + i
                    nc.tensor.matmul(pss[i], wt3[:, 2 * kp:2 * kp + 2, m * P:(m + 1) * P],
                                     rhs3[:, 2 * kp:2 * kp + 2, :],
                                     start=(kp == 0), stop=(kp == nk // 2 - 1),
                                     perf_mode=DR)
            for i in range(2):
                nc.vector.tensor_copy(outs[mg + i], pss[i])
        return outs

    def proj_v_early(wt3, kv3, nkv, nm):
        v_aug = [persist.tile([P, H, DV + 1], BF, name=f"pe_va{nm}{c}")
                 for c in range(CC)]
        for cg in range(0, CC, 2):
            pvs = [ps.tile([P, H * DV], F32, name=f"pe_pv{nm}{cg+i}", tag="b", bufs=3)
                   for i in range(2)]
            for kp in range(nkv // 2):
                for i in range(2):
                    c = cg + i
                    nc.tensor.matmul(pvs[i], kv3[:, 2 * kp:2 * kp + 2, c * P:(c + 1) * P],
                                     wt3[:, 2 * kp:2 * kp + 2, :],
                                     start=(kp == 0), stop=(kp == nkv // 2 - 1),
                                     perf_mode=DR)
            for i in range(2):
                c = cg + i
                nc.vector.tensor_copy(v_aug[c][:, :, 0:DV],
                                      pvs[i].rearrange("p (h d) -> p h d", h=H))
                nc.gpsimd.memset(v_aug[c][:, :, DV:DV + 1], 1.0)
        return v_aug

    stageA('q'); stageA('e')
    pre_q0 = proj_early(wls[0]['sWq'], KE83, DC, "q0")
    pre_k0 = proj_early(wls[0]['sWk'], KE83, DC, "k0")
    Spart = [s2q.tile([P, LC], BF, name=f"Spart{mc}") for mc in range(DC)]
    for mc in range(DC):
        psp = ps.tile([P, LC], F32, tag="a", bufs=3)
        for k in range(DC):
            nc.tensor.matmul(psp, cqa_WT[k][:, mc * P:(mc + 1) * P], S_T[k],
                             start=(k == 0), stop=(k == DC - 1))
        nc.scalar.activation(Spart[mc], psp, AF.Identity, bias=0.0,
                             scale=SW * SXB)

    stageB('q'); stageB('e')
    pre_v0 = proj_v_early(wls[0]['sWv'], KE83, DC, "v0")
    stageC('q', 0); stageC('e', DC)
    pools['e'].release(); pools['q'].release()
    cqaw.release(); s2q.release()

    # ---------------- phase 2: knowledge attention stack ----------------
    mp = ctx.enter_context(tc.tile_pool(name="mp", bufs=1))
    wl1pool = tc.alloc_tile_pool(name="wl1", bufs=1)
    wls[1] = alloc_wl(1, wl1pool, ('sWq', 'sWk', 'sWv', 'sWfc', 'cWq', 'cWfc'))
    wc0 = alloc_wl(0, wl1pool, ('cWq', 'cWfc'))

    def proj(wt3, rhs3, nk, out_name, tagbase):
        outs = [mp.tile([P, LK], BF, name=f"{out_name}{m}", tag=f"{tagbase}{m}",
                        bufs=1) for m in range(4)]
        for mg in range(0, 4, 2):
            pss = [ps.tile([P, LK], F32, name=f"pss{mg+i}", tag="a", bufs=3)
                   for i in range(2)]
            for kp in range(nk // 2):
                for i in range(2):
                    m = mg + i
                    nc.tensor.matmul(pss[i], wt3[:, 2 * kp:2 * kp + 2, m * P:(m + 1) * P],
                                     rhs3[:, 2 * kp:2 * kp + 2, :],
                                     start=(kp == 0), stop=(kp == nk // 2 - 1),
                                     perf_mode=DR)
            for i in range(2):
                nc.vector.tensor_copy(outs[mg + i], pss[i])
        return outs

    def proj_stream(w_dram, rhs3, nk, out_name, tagbase, wpool):
        outs = [mp.tile([P, LK], BF, name=f"{out_name}{m}", tag=f"{tagbase}{m}",
                        bufs=2) for m in range(4)]
        GS = 4
        wts = []
        for k0 in range(0, nk, GS):
            wt3 = wpool.tile([P, GS, H * DK], E4, name=f"w{out_name}{k0}",
                             tag="wst", bufs=3)
            nc.sync.dma_start(out=wt3, in_=w_dram[:, k0 * H * DK:(k0 + GS) * H * DK])
            wts.append(wt3)
        for mg in range(0, 4, 2):
            pss = [ps.tile([P, LK], F32, name=f"h{out_name}{mg+i}", tag="h", bufs=2)
                   for i in range(2)]
            for k0 in range(0, nk, GS):
                wt3 = wts[k0 // GS]
                for k in range(0, GS, 2):
                    for i in range(2):
                        m = mg + i
                        nc.tensor.matmul(pss[i], wt3[:, k:k + 2, m * P:(m + 1) * P],
                                         rhs3[:, k0 + k:k0 + k + 2, :],
                                         start=(k0 + k == 0), stop=(k0 + k == nk - 2),
                                         perf_mode=DR)
            for i in range(2):
                nc.vector.tensor_copy(outs[mg + i], pss[i])
        return outs

    def proj_v_stream(w_dram, kv3, nkv, tag, wpool):
        v_aug = [mp.tile([P, H, DV + 1], BF, name=f"va{tag}{c}", tag=f"va{tag}{c}",
                         bufs=1) for c in range(CC)]
        GS = 4
        wts = []
        for k0 in range(0, nkv, GS):
            wt3 = wpool.tile([P, GS, H * DV], E4, name=f"wv{tag}{k0}",
                             tag="wst", bufs=3)
            nc.sync.dma_start(out=wt3, in_=w_dram[:, k0 * H * DV:(k0 + GS) * H * DV])
            wts.append(wt3)
        for cg in range(0, CC, 2):
            pvs = [ps.tile([P, H * DV], F32, name=f"hv{tag}{cg+i}", tag="h", bufs=2)
                   for i in range(2)]
            for k0 in range(0, nkv, GS):
                wt3 = wts[k0 // GS]
                for k in range(0, GS, 2):
                    for i in range(2):
                        c = cg + i
                        nc.tensor.matmul(pvs[i], kv3[:, k0 + k:k0 + k + 2, c * P:(c + 1) * P],
                                         wt3[:, k:k + 2, :],
                                         start=(k0 + k == 0), stop=(k0 + k == nkv - 2),
                                         perf_mode=DR)
            for i in range(2):
                c = cg + i
                nc.vector.tensor_copy(v_aug[c][:, :, 0:DV],
                                      pvs[i].rearrange("p (h d) -> p h d", h=H))
                nc.gpsimd.memset(v_aug[c][:, :, DV:DV + 1], 1.0)
        return v_aug

    def proj_v(wt3, kv3, nkv, tag):
        v_aug = [mp.tile([P, H, DV + 1], BF, name=f"va{tag}{c}", tag=f"va{tag}{c}",
                         bufs=1) for c in range(CC)]
        for cg in range(0, CC, 2):
            pvs = [ps.tile([P, H * DV], F32, name=f"pvs{cg+i}", tag="a", bufs=3)
                   for i in range(2)]
            for kp in range(nkv // 2):
                for i in range(2):
                    c = cg + i
                    nc.tensor.matmul(pvs[i], kv3[:, 2 * kp:2 * kp + 2, c * P:(c + 1) * P],
                                     wt3[:, 2 * kp:2 * kp + 2, :],
                                     start=(kp == 0), stop=(kp == nkv // 2 - 1),
                                     perf_mode=DR)
            for i in range(2):
                c = cg + i
                nc.vector.tensor_copy(v_aug[c][:, :, 0:DV],
                                      pvs[i].rearrange("p (h d) -> p h d", h=H))
                nc.gpsimd.memset(v_aug[c][:, :, DV:DV + 1], 1.0)
        return v_aug

    def mha_ln(x_T, x3, w, pre, g_ap, b_ap, tag, out_f32=False, pre_q=None,
               skip_ln=False, dma_row0=None):
        """x_T: 6 [P,LK] bf16 query-side tiles; x3: same data as [P,DC,LK] fp8.
        w: (wq3, wfc3) fp8. pre: (k_T, v_aug). returns (y, y8)."""
        wq3, wfc3 = w
        k_T, v_aug = pre
        q_T = pre_q if pre_q is not None else proj(wq3, x3, DC, f"q{tag}", "qT")
        o8 = mp.tile([P, 4, LK], E4, name=f"o8{tag}", tag="o8", bufs=1)
        for g in range(2):
            povs = []
            for hh in range(4):
                h = g * 4 + hh
                t, o = h // 2, (h % 2) * DK
                e_sb = []
                for c in range(CC):
                    pa = ps.tile([P, LK], F32, tag="a", bufs=3)
                    nc.tensor.matmul(pa, k_T[t][o:o + DK, c * P:(c + 1) * P],
                                     q_T[t][o:o + DK, :], start=True, stop=True)
                    es = mp.tile([P, LK], BF, name=f"es{tag}{h}{c}", tag="es", bufs=8)
                    nc.scalar.activation(es, pa, AF.Exp, scale=SCALE / (SW * SW))
                    e_sb.append(es)
                pov = ps.tile([DV + 1, LK], F32, tag="b", bufs=3)
                for c in range(CC):
                    nc.tensor.matmul(pov, v_aug[c][:, h, :], e_sb[c],
                                     start=(c == 0), stop=(c == CC - 1))
                povs.append(pov)
            for hh in range(4):
                h = g * 4 + hh
                t, o = h // 2, (h % 2) * DK
                rrs = mp.tile([1, LK], F32, name=f"rrs{tag}{h}", tag="rrs", bufs=2)
                nc.vector.tensor_copy(rrs, povs[hh][DV:DV + 1, :])
                rr = mp.tile([1, LK], F32, name=f"rr{tag}{h}", tag="rr", bufs=2)
                nc.vector.reciprocal_approx_fast(out=rr, in_=rrs)
                pbc = mp.tile([DV, LK], F32, name=f"pbc{tag}{h}", tag="pbc", bufs=2)
                nc.gpsimd.partition_broadcast(pbc, rr)
                nc.vector.tensor_tensor(o8[o:o + DK, t, :], povs[hh][:DV, :],
                                        pbc, op=MUL)
        # --- fc + residual + LN ---
        x1 = [mp.tile([P, LK], BF, name=f"x1{tag}{d}", tag=f"x1{d}", bufs=1)
              for d in range(DC)]
        for d in range(DC):
            pf = ps.tile([P, LK], F32, tag="a", bufs=3)
            for kp in range(2):
                nc.tensor.matmul(pf, wfc3[:, 2 * kp:2 * kp + 2, d * P:(d + 1) * P],
                                 o8[:, 2 * kp:2 * kp + 2, :],
                                 start=(kp == 0), stop=(kp == 1), perf_mode=DR)
            nc.vector.scalar_tensor_tensor(x1[d], pf, 1.0 / (SW * SW), x_T[d],
                                           op0=MUL, op1=ADD)
            if skip_ln and dma_row0 is not None:
                nc.sync.dma_start(out=out_t[(dma_row0 + d) * P:
                                            (dma_row0 + d + 1) * P, :],
                                  in_=x1[d])
        if skip_ln:
            return x1, None
        ps_s = ps.tile([1, LK], F32, tag="b", bufs=3)
        ps_q = ps.tile([1, LK], F32, tag="b", bufs=3)
        sqs = [mp.tile([P, LK], BF, name=f"sq{tag}{d}", tag="sq", bufs=3)
               for d in range(DC)]
        for d in range(DC):
            eng = nc.vector if d % 2 == 0 else nc.gpsimd
            eng.tensor_tensor(sqs[d], x1[d], x1[d], op=MUL)
        for d in range(DC):
            nc.tensor.matmul(ps_s, ones_col, x1[d], start=(d == 0), stop=(d == DC - 1))
        for d in range(DC):
            nc.tensor.matmul(ps_q, ones_col, sqs[d], start=(d == 0), stop=(d == DC - 1))
        mu = mp.tile([1, LK], F32, name=f"mu{tag}", tag="mu", bufs=1)
        nc.scalar.activation(mu, ps_s, AF.Copy, bias=0.0, scale=1.0 / D)
        msq = mp.tile([1, LK], F32, name=f"msq{tag}", tag="msq", bufs=1)
        nc.scalar.activation(msq, ps_q, AF.Copy, bias=0.0, scale=1.0 / D)
        var = mp.tile([1, LK], F32, name=f"var{tag}", tag="var", bufs=1)
        nc.vector.tensor_tensor(var, mu, mu, op=MUL)
        nc.vector.tensor_tensor(var, msq, var, op=SUB)
        lv = mp.tile([1, LK], F32, name=f"lv{tag}", tag="lv", bufs=1)
        nc.scalar.activation(lv, var, AF.Ln, bias=eps_t, scale=1.0)
        rstd = mp.tile([1, LK], F32, name=f"rstd{tag}", tag="rstd", bufs=1)
        nc.scalar.activation(rstd, lv, AF.Exp, bias=0.0, scale=-0.5)
        c2 = mp.tile([1, LK], F32, name=f"c2{tag}", tag="c2", bufs=1)
        nc.vector.tensor_tensor(c2, mu, rstd, op=MUL)
        rstdb = mp.tile([1, LK], BF, name=f"rstdb{tag}", tag="rstdb", bufs=1)
        nc.vector.tensor_copy(rstdb, rstd)
        c2b = mp.tile([1, LK], BF, name=f"c2b{tag}", tag="c2b", bufs=1)
        nc.vector.tensor_copy(c2b, c2)
        pA = ps.tile([P, LK], F32, tag="b", bufs=3)
        nc.tensor.matmul(pA, ones_row, rstdb, start=True, stop=True)
        pC = ps.tile([P, LK], F32, tag="b", bufs=3)
        nc.tensor.matmul(pC, ones_row, c2b, start=True, stop=True)
        y = [mp.tile([P, LK], BF, name=f"y{tag}{d}", tag=f"y{tag[0]}{d}", bufs=1)
             for d in range(DC)]
        y8 = mp.tile([P, DC, LK], E4, name=f"y8{tag}", tag=f"y8{tag[0]}", bufs=1)
        for d in range(DC):
            nc.vector.tensor_tensor(y[d], x1[d], pA, op=MUL)
            nc.vector.tensor_tensor(y[d], y[d], pC, op=SUB)
            if d % 2 == 1:
                nc.gpsimd.tensor_copy(y8[:, d, :], y[d])
            else:
                nc.scalar.activation(y8[:, d, :], y[d], AF.Identity,
                                     bias=b_ap[d], scale=g_ap[d])

        return y, y8

    cur, cur8 = ke_T, KE83
    pre_c = None
    for l in range(NL):
        w = dict(wls[l])
        if l == 0:
            w.update(wc0)
        g1 = [lnv[l][:, d, 0:1] for d in range(DC)]
        b1 = [lnv[l][:, d, 1:2] for d in range(DC)]
        g2 = [lnv[l][:, d, 2:3] for d in range(DC)]
        b2 = [lnv[l][:, d, 3:4] for d in range(DC)]
        if l == 0:
            # hoisted: cross-attn k/v depend only on att (ready after phase 1)
            wstr = tc.alloc_tile_pool(name=f"wstr{l}", bufs=1)
            pre_c = (proj_stream(dt[f'cWk{l}'], att8, 2 * DC, f"kc{l}", "kTc", wstr),
                     proj_v_stream(dt[f'cWv{l}'], att8, 2 * DC, f"c{l}", wstr))
            wstr.release()
            dma_wl(1, wls[1])
            dma_wl(0, wc0)
        if l == 0:
            pre_s, pq = (pre_k0, pre_v0), pre_q0
        else:
            pre_s = (proj(w['sWk'], cur8, DC, f"ks{l}", "kTs"),
                     proj_v(w['sWv'], cur8, DC, f"s{l}"))
            pq = None
        so, so8 = mha_ln(cur, cur8, (w['sWq'], w['sWfc']), pre_s, g1, b1,
                         f"s{l}", pre_q=pq)
        if l + 1 < NL:
            # next layer's cross-kv hoists: K fills the s-attn scalar stalls,
            # V fills the c-attn scalar stalls (both only need att8)
            wstrk = tc.alloc_tile_pool(name=f"wstrk{l+1}", bufs=1)
            k_next = proj_stream(dt[f'cWk{l+1}'], att8, 2 * DC, f"kc{l+1}", "kTc", wstrk)
            wstrk.release()
            wstrv = tc.alloc_tile_pool(name=f"wstrv{l+1}", bufs=1)
            v_next = proj_v_stream(dt[f'cWv{l+1}'], att8, 2 * DC, f"c{l+1}", wstrv)
            wstrv.release()
            pre_c_next = (k_next, v_next)
        cur, cur8 = mha_ln(so, so8, (w['cWq'], w['cWfc']), pre_c, g2, b2, f"c{l}",
                     out_f32=(l == NL - 1), skip_ln=(l == NL - 1),
                     dma_row0=(2 * DC if l == NL - 1 else None))
        if l + 1 < NL:
            pre_c = pre_c_next
    wl1pool.release()
    ctx.close()


def _t128(a):
    # [n*128, w] -> [128, n*w] so each partition's DMA line is contiguous
    n = a.shape[0] // P
    return np.ascontiguousarray(
        a.reshape(n, P, a.shape[1]).transpose(1, 0, 2).reshape(P, -1))


def _t128pad(a):
    # ragged rows (LQ=160): pad to 2*128 rows then tile
    out = np.zeros((2 * P, a.shape[1]), a.dtype)
    out[:a.shape[0]] = a
    return _t128(out)


def kernel(**inputs):
    if 'nc' not in _CACHE:
        _CACHE['nc'] = _build()
    nc = _CACHE['nc']
    f = lambda x: np.ascontiguousarray(np.asarray(x), dtype=np.float32)
    bf = lambda x: np.asarray(x, dtype=np.float32).astype(NPBF)
    bfT = lambda x: np.asarray(x, dtype=np.float32).T.astype(NPBF)
    f8w = lambda x: np.clip(np.asarray(x, np.float32) * SW,
                            -240, 240).astype(NPE4)
    seq = f(inputs['sequences']); qry = f(inputs['query']); evd = f(inputs['evidence'])
    ke = f(inputs['knowledge_embed'])
    vecs = _t128(np.ascontiguousarray(np.stack(
        [f(inputs['w4C'])[:, 0], f(inputs['w4Q'])[:, 0],
         f(inputs['w4mlu'])[0, 0, :], f(inputs['cqa_b'])], axis=1)))
    cqa_WTf = np.ascontiguousarray(np.asarray(inputs['cqa_W'], np.float32).T)
    cqa_WTs = _t128(cqa_WTf[:D].astype(NPBF))
    cqa_WT8 = _t128(f8w(cqa_WTf[D:]))
    lwb = {}
    for l in range(NL):
        lwb[f'sWq{l}'] = _t128(f8w(inputs['L_sWq'][l]))
        lwb[f'sWk{l}'] = _t128(f8w(inputs['L_sWk'][l]))
        lwb[f'sWv{l}'] = _t128(f8w(inputs['L_sWv'][l]))
        lwb[f'sWfc{l}'] = _t128(f8w(inputs['L_sWfc'][l]))
        lwb[f'cWq{l}'] = _t128(f8w(inputs['L_cWq'][l]))
        lwb[f'cWk{l}'] = _t128(f8w(inputs['L_cWk'][l]))
        lwb[f'cWv{l}'] = _t128(f8w(inputs['L_cWv'][l]))
        lwb[f'cWfc{l}'] = _t128(f8w(inputs['L_cWfc'][l]))
        lwb[f'ln{l}'] = _t128(np.ascontiguousarray(np.stack(
            [f(inputs['L_n1g'][l]), f(inputs['L_n1b'][l]),
             f(inputs['L_n2g'][l]), f(inputs['L_n2b'][l])], axis=1)))
    in_maps = []
    for b in range(B):
        keT = np.ascontiguousarray(ke[b].T)
        m = {
            'S_nat': _t128(bf(seq[b])), 'S_T': _t128(bfT(seq[b])),
            'Q_nat': _t128pad(bf(qry[b])), 'Q_T': _t128(bfT(qry[b])),
            'E_nat': _t128pad(bf(evd[b])), 'E_T': _t128(bfT(evd[b])),
            'KE_T': _t128(keT.astype(NPBF)),
            'KE8': _t128(np.clip(keT, -240, 240).astype(NPE4)),
            'vecs': vecs, 'cqa_WTs': cqa_WTs, 'cqa_WT8': cqa_WT8,
        }
        m.update(lwb)
        in_maps.append(m)
    _CACHE['last_in_maps'] = in_maps
    res = run_bass_kernel_spmd(nc, in_maps, core_ids=list(range(B)))
    _CACHE['last_results'] = res
    outs = np.stack([np.asarray(r['out_t'], dtype=np.float32)
                     for r in res.results])                      # [B, 2304, 512]
    full = outs.transpose(0, 2, 1)                               # [B, 512, 2304]
    x1 = full[:, :, 2 * D:]                                      # pre-LN ke [B,512,768]
    muh = x1.mean(-1, keepdims=True)
    varh = x1.var(-1, keepdims=True)
    g = np.asarray(inputs['L_n2g'][NL - 1], dtype=np.float32)
    bb = np.asarray(inputs['L_n2b'][NL - 1], dtype=np.float32)
    ke_out = (x1 - muh) / np.sqrt(varh + EPS) * g + bb
    out = np.concatenate([seq, full[:, :, :2 * D], ke_out], axis=-1)
    return out



# revision 40
# speedup vs baseline: 1.1243x; 1.1243x over previous
import sys
if '/opt/trn_rl_repo' not in sys.path:
    sys.path.insert(0, '/opt/trn_rl_repo')
import numpy as np
import ml_dtypes

import concourse.bass as bass
import concourse.bacc as bacc
import concourse.tile as tile
from concourse import mybir
from concourse.bass_utils import run_bass_kernel_spmd
from concourse.masks import make_identity

F32 = mybir.dt.float32
BF = mybir.dt.bfloat16
E4 = mybir.dt.float8e4
AF = mybir.ActivationFunctionType
MUL = mybir.AluOpType.mult
ADD = mybir.AluOpType.add
SUB = mybir.AluOpType.subtract
DR = mybir.MatmulPerfMode.DoubleRow
P = 128
NPE4 = ml_dtypes.float8_e4m3
SW = 64.0     # fp8 weight pre-scale (host); descale folded into exp/fc/cqa
SXB = 8.0     # fp8 cqa xblock pre-scale (device)
D, H, DK, DV, NL = 768, 8, 64, 64, 2
B, LC, LQ, LK = 8, 512, 160, 512
DC = D // P      # 6 chunks of the 768 dim
CC = LC // P     # 4 chunks of the 512 token dim
QCH = [(0, 128), (128, 32)]   # (offset, size) chunks of LQ=160
SCALE = 0.125    # log_512(512)/sqrt(64)
EPS = 1e-6
NPBF = ml_dtypes.bfloat16

USE_SPART = True
USE_APPROX_RECIP = True
USE_GPS_BCAST = True
_CACHE = {}


def _build():
    nc = bacc.Bacc()
    dt = {}

    def din(name, shape, dtype=BF):
        dt[name] = nc.dram_tensor(name, list(shape), dtype, kind="ExternalInput")
        return dt[name]

    # all big tensors pre-tiled on host to [128, chunks*width] (partition-contiguous)
    din('S_nat', (P, CC * D)); din('S_T', (P, DC * LC))
    din('Q_nat', (P, 2 * D)); din('Q_T', (P, DC * LQ))
    din('E_nat', (P, 2 * D)); din('E_T', (P, DC * LQ))
    din('KE_T', (P, DC * LK)); din('KE8', (P, DC * LK), E4)
    din('vecs', (P, DC * 4), F32)    # cols: w4C, w4Q, w4mlu, cqa_b
    din('cqa_WTs', (P, DC * D))
    din('cqa_WT8', (P, 3 * DC * D), E4)
    for l in range(NL):
        din(f'sWq{l}', (P, DC * H * DK), E4); din(f'sWk{l}', (P, DC * H * DK), E4)
        din(f'sWv{l}', (P, DC * H * DV), E4); din(f'sWfc{l}', (P, 4 * D), E4)
        din(f'cWq{l}', (P, DC * H * DK), E4); din(f'cWk{l}', (P, 2 * DC * H * DK), E4)
        din(f'cWv{l}', (P, 2 * DC * H * DV), E4); din(f'cWfc{l}', (P, 4 * D), E4)
        din(f'ln{l}', (P, DC * 4), F32)   # cols: n1g, n1b, n2g, n2b
    out_t = nc.dram_tensor('out_t', [3 * D, LC], BF, kind="ExternalOutput")

    with tile.TileContext(nc) as tc:
        _emit(nc, tc, dt, out_t)
    nc.compile()
    return nc


def _emit(nc, tc, dt, out_t):
    from contextlib import ExitStack
    ctx = ExitStack()
    const = ctx.enter_context(tc.tile_pool(name="const", bufs=1))
    persist = ctx.enter_context(tc.tile_pool(name="persist", bufs=1))

    ident = const.tile([P, P], BF)
    make_identity(nc, ident)
    ones_row = const.tile([1, P], BF)
    nc.gpsimd.memset(ones_row, 1.0)
    ones_col = const.tile([P, 1], BF)
    nc.gpsimd.memset(ones_col, 1.0)
    eps_t = const.tile([1, 1], F32)
    nc.gpsimd.memset(eps_t, EPS)


    # ---- weight pool for layer 0 (tiles alloc'd early for LIFO order;
    # DMAs emitted after the input DMAs) ----
    def alloc_wl(l, wl, names):
        w = {}
        for nm in names:
            if nm in ('sWfc', 'cWfc'):
                w[nm] = wl.tile([P, 4, D], E4, name=f"{nm}{l}")
            elif nm == 'sWv':
                w[nm] = wl.tile([P, DC, H * DV], E4, name=f"{nm}{l}")
            else:
                w[nm] = wl.tile([P, DC, H * DK], E4, name=f"{nm}{l}")
        return w

    def dma_wl(l, w, names=None):
        for nm in (names if names is not None else w):
            nc.sync.dma_start(out=w[nm], in_=dt[f'{nm}{l}'][:, :])

    wl0pool = ctx.enter_context(tc.tile_pool(name="wl0", bufs=1))
    wls = [alloc_wl(0, wl0pool, ('sWq', 'sWk', 'sWv', 'sWfc')), None]
    ps = ctx.enter_context(tc.tile_pool(name="ps", bufs=1, space="PSUM"))

    # ---- input DMAs (batched; earliest-needed first) ----
    s2q = tc.alloc_tile_pool(name="s2q", bufs=1)
    cqaw = tc.alloc_tile_pool(name="cqaw", bufs=1)

    vecs = const.tile([P, DC, 4], F32)
    nc.sync.dma_start(out=vecs, in_=dt['vecs'][:, :])
    vecs_bf = const.tile([P, DC, 4], BF)
    nc.vector.tensor_copy(vecs_bf, vecs)

    ST3 = s2q.tile([P, DC, LC], BF, name="ST3")
    nc.sync.dma_start(out=ST3[:, 0:3, :], in_=dt['S_T'][:, 0:3 * LC])
    nc.sync.dma_start(out=ST3[:, 3:6, :], in_=dt['S_T'][:, 3 * LC:])
    S_T = [ST3[:, d, :] for d in range(DC)]

    qe_in = {}
    for tag, QN, QT in (('q', dt['Q_nat'], dt['Q_T']), ('e', dt['E_nat'], dt['E_T'])):
        qt3 = s2q.tile([P, DC, LQ], BF, name=f"Qt3{tag}")
        nc.sync.dma_start(out=qt3[:, 0:3, :], in_=QT[:, 0:3 * LQ])
        nc.sync.dma_start(out=qt3[:, 3:6, :], in_=QT[:, 3 * LQ:])
        qn3 = s2q.tile([P, 2, D], BF, name=f"Qn3{tag}")
        nc.sync.dma_start(out=qn3, in_=QN[:, :])
        Qn = [qn3[:, 0, :], qn3[:, 1, :]]
        qe_in[tag] = ([qt3[:, d, :] for d in range(DC)], Qn)

    KE3 = persist.tile([P, DC, LK], BF, name="KE3")
    nc.sync.dma_start(out=KE3, in_=dt['KE_T'][:, :])
    ke_T = [KE3[:, d, :] for d in range(DC)]
    KE83 = persist.tile([P, DC, LK], E4, name="KE83")
    nc.sync.dma_start(out=KE83, in_=dt['KE8'][:, :])

    dma_wl(0, wls[0], ('sWq', 'sWk'))

    SN3 = s2q.tile([P, CC, D], BF, name="SN3")
    nc.sync.dma_start(out=SN3, in_=dt['S_nat'][:, :])
    S_nat = [SN3[:, c, :] for c in range(CC)]

    dma_wl(0, wls[0], ('sWv',))

    CQ3s = cqaw.tile([P, DC, D], BF, name="CQ3s")
    nc.sync.dma_start(out=CQ3s, in_=dt['cqa_WTs'][:, :])
    cqa_WT = [CQ3s[:, k, :] for k in range(DC)]
    CQ8 = cqaw.tile([P, 3 * DC, D], E4, name="CQ8")
    nc.sync.dma_start(out=CQ8, in_=dt['cqa_WT8'][:, :])

    dma_wl(0, wls[0], ('sWfc',))

    lnv = []
    for l in range(NL):
        t = const.tile([P, DC, 4], F32, name=f"lnv{l}")
        nc.sync.dma_start(out=t, in_=dt[f'ln{l}'][:, :])
        lnv.append(t)

    # ---- S-side shared prep ----
    cm3 = s2q.tile([P, DC, LC], BF, name="cm3")
    cm_T = [cm3[:, d, :] for d in range(DC)]
    for d in range(DC):
        nc.vector.tensor_scalar_mul(cm_T[d], S_T[d], vecs[:, d, 2:3])
    ps0 = ps.tile([1, LC], F32, tag="b", bufs=3)
    for d in range(DC):
        nc.tensor.matmul(ps0, vecs_bf[:, d, 0:1], S_T[d], start=(d == 0), stop=(d == DC - 1))
    s0_row = s2q.tile([1, LC], BF)
    nc.vector.tensor_copy(s0_row, ps0)
    # att held in SBUF (fp8) for phase 2 cross-attn kv
    att8 = persist.tile([P, 2 * DC, LC], E4, name="att8")

    # ---- s2q in stages; q/e interleaved ----
    pools = {}
    st = {}

    def stageA(tag):
        po = tc.alloc_tile_pool(name=f"s2qt_{tag}", bufs=1)
        pools[tag] = po
        Qt, Qn = qe_in[tag]
        s1 = []
        for qi, (qo, qs) in enumerate(QCH):
            pq = ps.tile([P, 1], F32, tag="b", bufs=3)
            for d in range(DC):
                nc.tensor.matmul(pq[:qs], Qt[d][:, qo:qo + qs], vecs_bf[:, d, 1:2],
                                 start=(d == 0), stop=(d == DC - 1))
            t = po.tile([P, 1], F32, name=f"s1{tag}{qi}")
            nc.vector.tensor_copy(t[:qs], pq[:qs])
            s1.append(t)
        e_t, etn = [], []
        for qi, (qo, qs) in enumerate(QCH):
            psc_t = ps.tile([P, LC], F32, tag="a", bufs=3)
            for d in range(DC):
                nc.tensor.matmul(psc_t[:qs], Qt[d][:, qo:qo + qs], cm_T[d],
                                 start=(d == 0), stop=False)
            nc.tensor.matmul(psc_t[:qs], ones_row[:1, :qs], s0_row,
                             start=False, stop=True)
            et = po.tile([P, LC], BF, name=f"et{tag}{qi}")
            stt = po.tile([P, 1], F32, name=f"st{tag}{qi}")
            nc.scalar.activation(et[:qs], psc_t[:qs], AF.Exp, bias=s1[qi][:qs],
                                 scale=1.0, accum_out=stt[:qs])
            rt = po.tile([P, 1], F32, name=f"rt{tag}{qi}")
            nc.vector.reciprocal_approx_fast(out=rt[:qs], in_=stt[:qs])
            en = po.tile([P, LC], BF, name=f"etn{tag}{qi}")
            nc.vector.tensor_scalar_mul(en[:qs], et[:qs], rt[:qs])
            e_t.append(et); etn.append(en)
        psr = ps.tile([1, LC], F32, tag="b", bufs=3)
        for qi, (qo, qs) in enumerate(QCH):
            nc.tensor.matmul(psr, ones_col[:qs, :1], e_t[qi][:qs],
                             start=(qi == 0), stop=(qi == 1))
        rc_row = po.tile([1, LC], F32, name=f"rc{tag}")
        nc.scalar.copy(rc_row, psr)
        nc.vector.reciprocal_approx_fast(out=rc_row, in_=rc_row)
        st[tag] = dict(e_t=e_t, etn=etn, rc_row=rc_row)

    def stageB(tag):
        po = pools[tag]
        s = st[tag]
        rcb = po.tile([1, LC], BF, name=f"rcb{tag}")
        nc.vector.tensor_copy(rcb, s['rc_row'])
        pbs = ps.tile([P, LC], F32, tag="b", bufs=3)
        nc.tensor.matmul(pbs, ones_row, rcb, start=True, stop=True)
        P_T = []
        for qi, (qo, qs) in enumerate(QCH):
            pt = po.tile([P, LC], BF, name=f"PT{tag}{qi}")
            nc.vector.tensor_tensor(pt[:qs], s['e_t'][qi][:qs], pbs[:qs], op=MUL)
            P_T.append(pt)
        etn_T = [po.tile([P, LQ], BF, name=f"etnT{tag}{c}") for c in range(CC)]
        for c in range(CC):
            for qi, (qo, qs) in enumerate(QCH):
                pt = ps.tile([P, P], BF, tag="b", bufs=3)
                nc.tensor.transpose(pt[:, :qs], s['etn'][qi][:qs, c * P:(c + 1) * P],
                                    ident[:qs, :qs])
                nc.vector.tensor_copy(etn_T[c][:, qo:qo + qs], pt[:, :qs])
        tmp = []
        for qi, (qo, qs) in enumerate(QCH):
            t = po.tile([P, D], BF, name=f"tmp{tag}{qi}")
            for n in range(2):
                pm = ps.tile([P, 384], F32, tag="a", bufs=3)
                for c in range(CC):
                    nc.tensor.matmul(pm[:qs], etn_T[c][:, qo:qo + qs],
                                     S_nat[c][:, n * 384:(n + 1) * 384],
                                     start=(c == 0), stop=(c == CC - 1))
                nc.vector.tensor_copy(t[:qs, n * 384:(n + 1) * 384], pm[:qs])
            tmp.append(t)
        s['P_T'] = P_T; s['tmp'] = tmp

    def stageC(tag, row0):
        po = pools[tag]
        s = st[tag]
        Qt, Qn = qe_in[tag]
        P_T, tmp = s['P_T'], s['tmp']
        xb8 = po.tile([P, 3 * DC, LC], E4, name=f"xb8{tag}")
        for d in range(DC):
            pc = ps.tile([P, LC], F32, tag="a", bufs=3)
            for qi, (qo, qs) in enumerate(QCH):
                nc.tensor.matmul(pc, Qn[qi][:qs, d * P:(d + 1) * P], P_T[qi][:qs],
                                 start=(qi == 0), stop=(qi == 1))
            nc.vector.tensor_scalar_mul(xb8[:, d, :], pc, SXB)
            nc.vector.scalar_tensor_tensor(xb8[:, DC + d, :], pc, SXB, S_T[d],
                                           op0=MUL, op1=MUL)
            pq2 = ps.tile([P, LC], F32, tag="a", bufs=3)
            for qi, (qo, qs) in enumerate(QCH):
                nc.tensor.matmul(pq2, tmp[qi][:qs, d * P:(d + 1) * P], P_T[qi][:qs],
                                 start=(qi == 0), stop=(qi == 1))
            nc.vector.scalar_tensor_tensor(xb8[:, 2 * DC + d, :], pq2, SXB,
                                           S_T[d], op0=MUL, op1=MUL)
        for mc in range(DC):
            pco = ps.tile([P, LC], F32, tag="a", bufs=3)
            nc.tensor.matmul(pco, ident, Spart[mc], start=True, stop=False,
                             skip_group_check=True)
            for j in range(3 * DC // 2):
                nc.tensor.matmul(pco, CQ8[:, 2 * j:2 * j + 2, mc * P:(mc + 1) * P],
                                 xb8[:, 2 * j:2 * j + 2, :], start=False,
                                 stop=(j == 3 * DC // 2 - 1), perf_mode=DR,
                                 skip_group_check=True)
            ob = po.tile([P, LC], BF, name=f"ob{tag}{mc}", tag="attb", bufs=2)
            nc.scalar.activation(ob, pco, AF.Identity,
                                 bias=vecs[:, mc, 3:4], scale=1.0 / (SW * SXB))
            nc.sync.dma_start(out=out_t[(row0 + mc) * P:(row0 + mc + 1) * P, :],
                              in_=ob)
            nc.vector.tensor_copy(att8[:, row0 + mc, :], ob)

    def proj_early(wt3, rhs3, nk, nm):
        outs = [persist.tile([P, LK], BF, name=f"pe_{nm}{m}") for m in range(4)]
        for mg in range(0, 4, 2):
            pss = [ps.tile([P, LK], F32, name=f"pe_ps{nm}{mg+i}", tag="b", bufs=3)
                   for i in range(2)]
            for kp in range(nk // 2):
                for i in range(2):
                    m = mg + i
                    nc.tensor.matmul(pss[i], wt3[:, 2 * kp:2 * kp + 2, m * P:(m + 1) * P],
                                     rhs3[:, 2 * kp:2 * kp + 2, :],
                                     start=(kp == 0), stop=(kp == nk // 2 - 1),
                                     perf_mode=DR)
            for i in range(2):
                nc.vector.tensor_copy(outs[mg + i], pss[i])
        return outs

    def proj_v_early(wt3, kv3, nkv, nm):
        v_aug = [persist.tile([P, H, DV + 1], BF, name=f"pe_va{nm}{c}")
                 for c in range(CC)]
        for cg in range(0, CC, 2):
            pvs = [ps.tile([P, H * DV], F32, name=f"pe_pv{nm}{cg+i}", tag="b", bufs=3)
                   for i in range(2)]
            for kp in range(nkv // 2):
                for i in range(2):
                    c = cg + i
                    nc.tensor.matmul(pvs[i], kv3[:, 2 * kp:2 * kp + 2, c * P:(c + 1) * P],
                                     wt3[:, 2 * kp:2 * kp + 2, :],
                                     start=(kp == 0), stop=(kp == nkv // 2 - 1),
                                     perf_mode=DR)
            for i in range(2):
                c = cg + i
                nc.vector.tensor_copy(v_aug[c][:, :, 0:DV],
                                      pvs[i].rearrange("p (h d) -> p h d", h=H))
                nc.gpsimd.memset(v_aug[c][:, :, DV:DV + 1], 1.0)
        return v_aug

    stageA('q'); stageA('e')
    pre_q0 = proj_early(wls[0]['sWq'], KE83, DC, "q0")
    pre_k0 = proj_early(wls[0]['sWk'], KE83, DC, "k0")
    Spart = [s2q.tile([P, LC], BF, name=f"Spart{mc}") for mc in range(DC)]
    for mc in range(DC):
        psp = ps.tile([P, LC], F32, tag="a", bufs=3)
        for k in range(DC):
            nc.tensor.matmul(psp, cqa_WT[k][:, mc * P:(mc + 1) * P], S_T[k],
                             start=(k == 0), stop=(k == DC - 1))
        nc.scalar.activation(Spart[mc], psp, AF.Identity, bias=0.0,
                             scale=SW * SXB)

    stageB('q'); stageB('e')
    pre_v0 = proj_v_early(wls[0]['sWv'], KE83, DC, "v0")
    stageC('q', 0); stageC('e', DC)
    pools['e'].release(); pools['q'].release()
    cqaw.release(); s2q.release()

    # ---------------- phase 2: knowledge attention stack ----------------
    mp = ctx.enter_context(tc.tile_pool(name="mp", bufs=1))
    wl1pool = tc.alloc_tile_pool(name="wl1", bufs=1)
    wls[1] = alloc_wl(1, wl1pool, ('sWq', 'sWk', 'sWv', 'sWfc', 'cWq', 'cWfc'))
    wc0 = alloc_wl(0, wl1pool, ('cWq', 'cWfc'))

    def proj(wt3, rhs3, nk, out_name, tagbase):
        outs = [mp.tile([P, LK], BF, name=f"{out_name}{m}", tag=f"{tagbase}{m}",
                        bufs=1) for m in range(4)]
        for mg in range(0, 4, 2):
            pss = [ps.tile([P, LK], F32, name=f"pss{mg+i}", tag="a", bufs=3)
                   for i in range(2)]
            for kp in range(nk // 2):
                for i in range(2):
                    m = mg + i
                    nc.tensor.matmul(pss[i], wt3[:, 2 * kp:2 * kp + 2, m * P:(m + 1) * P],
                                     rhs3[:, 2 * kp:2 * kp + 2, :],
                                     start=(kp == 0), stop=(kp == nk // 2 - 1),
                                     perf_mode=DR)
            for i in range(2):
                nc.vector.tensor_copy(outs[mg + i], pss[i])
        return outs

    def proj_stream(w_dram, rhs3, nk, out_name, tagbase, wpool):
        outs = [mp.tile([P, LK], BF, name=f"{out_name}{m}", tag=f"{tagbase}{m}",
                        bufs=2) for m in range(4)]
        GS = 4
        wts = []
        for k0 in range(0, nk, GS):
            wt3 = wpool.tile([P, GS, H * DK], E4, name=f"w{out_name}{k0}",
                             tag="wst", bufs=3)
            nc.sync.dma_start(out=wt3, in_=w_dram[:, k0 * H * DK:(k0 + GS) * H * DK])
            wts.append(wt3)
        for mg in range(0, 4, 2):
            pss = [ps.tile([P, LK], F32, name=f"h{out_name}{mg+i}", tag="h", bufs=2)
                   for i in range(2)]
            for k0 in range(0, nk, GS):
                wt3 = wts[k0 // GS]
                for k in range(0, GS, 2):
                    for i in range(2):
                        m = mg + i
                        nc.tensor.matmul(pss[i], wt3[:, k:k + 2, m * P:(m + 1) * P],
                                         rhs3[:, k0 + k:k0 + k + 2, :],
                                         start=(k0 + k == 0), stop=(k0 + k == nk - 2),
                                         perf_mode=DR)
            for i in range(2):
                nc.vector.tensor_copy(outs[mg + i], pss[i])
        return outs

    def proj_v_stream(w_dram, kv3, nkv, tag, wpool):
        v_aug = [mp.tile([P, H, DV + 1], BF, name=f"va{tag}{c}", tag=f"va{tag}{c}",
                         bufs=1) for c in range(CC)]
        GS = 4
        wts = []
        for k0 in range(0, nkv, GS):
            wt3 = wpool.tile([P, GS, H * DV], E4, name=f"wv{tag}{k0}",
                             tag="wst", bufs=3)
            nc.sync.dma_start(out=wt3, in_=w_dram[:, k0 * H * DV:(k0 + GS) * H * DV])
            wts.append(wt3)
        for cg in range(0, CC, 2):
            pvs = [ps.tile([P, H * DV], F32, name=f"hv{tag}{cg+i}", tag="h", bufs=2)
                   for i in range(2)]
            for k0 in range(0, nkv, GS):
                wt3 = wts[k0 // GS]
                for k in range(0, GS, 2):
                    for i in range(2):
                        c = cg + i
                        nc.tensor.matmul(pvs[i], kv3[:, k0 + k:k0 + k + 2, c * P:(c + 1) * P],
                                         wt3[:, k:k + 2, :],
                                         start=(k0 + k == 0), stop=(k0 + k == nkv - 2),
                                         perf_mode=DR)
            for i in range(2):
                c = cg + i
                nc.vector.tensor_copy(v_aug[c][:, :, 0:DV],
                                      pvs[i].rearrange("p (h d) -> p h d", h=H))
                nc.gpsimd.memset(v_aug[c][:, :, DV:DV + 1], 1.0)
        return v_aug

    def proj_v(wt3, kv3, nkv, tag):
        v_aug = [mp.tile([P, H, DV + 1], BF, name=f"va{tag}{c}", tag=f"va{tag}{c}",
                         bufs=1) for c in range(CC)]
        for cg in range(0, CC, 2):
            pvs = [ps.tile([P, H * DV], F32, name=f"pvs{cg+i}", tag="a", bufs=3)
                   for i in range(2)]
            for kp in range(nkv // 2):
                for i in range(2):
                    c = cg + i
                    nc.tensor.matmul(pvs[i], kv3[:, 2 * kp:2 * kp + 2, c * P:(c + 1) * P],
                                     wt3[:, 2 * kp:2 * kp + 2, :],
                                     start=(kp == 0), stop=(kp == nkv // 2 - 1),
                                     perf_mode=DR)
            for i in range(2):
                c = cg + i
                nc.vector.tensor_copy(v_aug[c][:, :, 0:DV],
                                      pvs[i].rearrange("p (h d) -> p h d", h=H))
                nc.gpsimd.memset(v_aug[c][:, :, DV:DV + 1], 1.0)
        return v_aug

    def mha_ln(x_T, x3, w, pre, g_ap, b_ap, tag, out_f32=False, pre_q=None,
               skip_ln=False, dma_row0=None):
        """x_T: 6 [P,LK] bf16 query-side tiles; x3: same data as [P,DC,LK] fp8.
        w: (wq3, wfc3) fp8. pre: (k_T, v_aug). returns (y, y8)."""
        wq3, wfc3 = w
        k_T, v_aug = pre
        q_T = pre_q if pre_q is not None else proj(wq3, x3, DC, f"q{tag}", "qT")
        o8 = mp.tile([P, 4, LK], E4, name=f"o8{tag}", tag="o8", bufs=1)
        for g in range(2):
            povs = []
            for hh in range(4):
                h = g * 4 + hh
                t, o = h // 2, (h % 2) * DK
                e_sb = []
                for c in range(CC):
                    pa = ps.tile([P, LK], F32, tag="a", bufs=3)
                    nc.tensor.matmul(pa, k_T[t][o:o + DK, c * P:(c + 1) * P],
                                     q_T[t][o:o + DK, :], start=True, stop=True)
                    es = mp.tile([P, LK], BF, name=f"es{tag}{h}{c}", tag="es", bufs=8)
                    nc.scalar.activation(es, pa, AF.Exp, scale=SCALE / (SW * SW))
                    e_sb.append(es)
                pov = ps.tile([DV + 1, LK], F32, tag="b", bufs=3)
                for c in range(CC):
                    nc.tensor.matmul(pov, v_aug[c][:, h, :], e_sb[c],
                                     start=(c == 0), stop=(c == CC - 1))
                povs.append(pov)
            for hh in range(4):
                h = g * 4 + hh
                t, o = h // 2, (h % 2) * DK
                rrs = mp.tile([1, LK], F32, name=f"rrs{tag}{h}", tag="rrs", bufs=2)
                nc.vector.tensor_copy(rrs, povs[hh][DV:DV + 1, :])
                rr = mp.tile([1, LK], F32, name=f"rr{tag}{h}", tag="rr", bufs=2)
                nc.vector.reciprocal_approx_fast(out=rr, in_=rrs)
                pbc = mp.tile([DV, LK], F32, name=f"pbc{tag}{h}", tag="pbc", bufs=2)
                nc.gpsimd.partition_broadcast(pbc, rr)
                nc.vector.tensor_tensor(o8[o:o + DK, t, :], povs[hh][:DV, :],
                                        pbc, op=MUL)
        # --- fc + residual + LN ---
        x1 = [mp.tile([P, LK], BF, name=f"x1{tag}{d}", tag=f"x1{d}", bufs=1)
              for d in range(DC)]
        for d in range(DC):
            pf = ps.tile([P, LK], F32, tag="a", bufs=3)
            for kp in range(2):
                nc.tensor.matmul(pf, wfc3[:, 2 * kp:2 * kp + 2, d * P:(d + 1) * P],
                                 o8[:, 2 * kp:2 * kp + 2, :],
                                 start=(kp == 0), stop=(kp == 1), perf_mode=DR)
            nc.vector.scalar_tensor_tensor(x1[d], pf, 1.0 / (SW * SW), x_T[d],
                                           op0=MUL, op1=ADD)
            if skip_ln and dma_row0 is not None:
                nc.sync.dma_start(out=out_t[(dma_row0 + d) * P:
                                            (dma_row0 + d + 1) * P, :],
                                  in_=x1[d])
        if skip_ln:
            return x1, None
        ps_s = ps.tile([1, LK], F32, tag="b", bufs=3)
        ps_q = ps.tile([1, LK], F32, tag="b", bufs=3)
        sqs = [mp.tile([P, LK], BF, name=f"sq{tag}{d}", tag="sq", bufs=3)
               for d in range(DC)]
        for d in range(DC):
            nc.vector.tensor_tensor(sqs[d], x1[d], x1[d], op=MUL)
        for d in range(DC):
            nc.tensor.matmul(ps_s, ones_col, x1[d], start=(d == 0), stop=(d == DC - 1))
        for d in range(DC):
            nc.tensor.matmul(ps_q, ones_col, sqs[d], start=(d == 0), stop=(d == DC - 1))
        mu = mp.tile([1, LK], F32, name=f"mu{tag}", tag="mu", bufs=1)
        nc.scalar.activation(mu, ps_s, AF.Copy, bias=0.0, scale=1.0 / D)
        msq = mp.tile([1, LK], F32, name=f"msq{tag}", tag="msq", bufs=1)
        nc.scalar.activation(msq, ps_q, AF.Copy, bias=0.0, scale=1.0 / D)
        var = mp.tile([1, LK], F32, name=f"var{tag}", tag="var", bufs=1)
        nc.vector.tensor_tensor(var, mu, mu, op=MUL)
        nc.vector.tensor_tensor(var, msq, var, op=SUB)
        lv = mp.tile([1, LK], F32, name=f"lv{tag}", tag="lv", bufs=1)
        nc.scalar.activation(lv, var, AF.Ln, bias=eps_t, scale=1.0)
        rstd = mp.tile([1, LK], F32, name=f"rstd{tag}", tag="rstd", bufs=1)
        nc.scalar.activation(rstd, lv, AF.Exp, bias=0.0, scale=-0.5)
        c2 = mp.tile([1, LK], F32, name=f"c2{tag}", tag="c2", bufs=1)
        nc.vector.tensor_tensor(c2, mu, rstd, op=MUL)
        rstdb = mp.tile([1, LK], BF, name=f"rstdb{tag}", tag="rstdb", bufs=1)
        nc.vector.tensor_copy(rstdb, rstd)
        c2b = mp.tile([1, LK], BF, name=f"c2b{tag}", tag="c2b", bufs=1)
        nc.vector.tensor_copy(c2b, c2)
        pA = ps.tile([P, LK], F32, tag="b", bufs=3)
        nc.tensor.matmul(pA, ones_row, rstdb, start=True, stop=True)
        pC = ps.tile([P, LK], F32, tag="b", bufs=3)
        nc.tensor.matmul(pC, ones_row, c2b, start=True, stop=True)
        y = [mp.tile([P, LK], BF, name=f"y{tag}{d}", tag=f"y{tag[0]}{d}", bufs=1)
             for d in range(DC)]
        y8 = mp.tile([P, DC, LK], E4, name=f"y8{tag}", tag=f"y8{tag[0]}", bufs=1)
        yt = [mp.tile([P, LK], BF, name=f"yt{tag}{d}", tag="yt", bufs=3)
              for d in range(DC)]
        for d in range(DC):
            nc.vector.tensor_tensor(yt[d], x1[d], pA, op=MUL)
            nc.vector.tensor_tensor(yt[d], yt[d], pC, op=SUB)
            if d % 2 == 1:
                nc.scalar.activation(y8[:, d, :], yt[d], AF.Identity,
                                     bias=b_ap[d], scale=g_ap[d])
            else:
                nc.vector.tensor_scalar(y8[:, d, :], yt[d], g_ap[d], b_ap[d],
                                        op0=MUL, op1=ADD)
        for d in range(DC):
            if d % 2 == 1:
                nc.vector.tensor_scalar(y[d], yt[d], g_ap[d], b_ap[d],
                                        op0=MUL, op1=ADD)
            else:
                nc.scalar.activation(y[d], yt[d], AF.Identity,
                                     bias=b_ap[d], scale=g_ap[d])

        return y, y8

    cur, cur8 = ke_T, KE83
    pre_c = None
    for l in range(NL):
        w = dict(wls[l])
        if l == 0:
            w.update(wc0)
        g1 = [lnv[l][:, d, 0:1] for d in range(DC)]
        b1 = [lnv[l][:, d, 1:2] for d in range(DC)]
        g2 = [lnv[l][:, d, 2:3] for d in range(DC)]
        b2 = [lnv[l][:, d, 3:4] for d in range(DC)]
        if l == 0:
            # hoisted: cross-attn k/v depend only on att (ready after phase 1)
            wstr = tc.alloc_tile_pool(name=f"wstr{l}", bufs=1)
            pre_c = (proj_stream(dt[f'cWk{l}'], att8, 2 * DC, f"kc{l}", "kTc", wstr),
                     proj_v_stream(dt[f'cWv{l}'], att8, 2 * DC, f"c{l}", wstr))
            wstr.release()
            dma_wl(1, wls[1])
            dma_wl(0, wc0)
        if l == 0:
            pre_s, pq = (pre_k0, pre_v0), pre_q0
        else:
            pre_s = (proj(w['sWk'], cur8, DC, f"ks{l}", "kTs"),
                     proj_v(w['sWv'], cur8, DC, f"s{l}"))
            pq = None
        so, so8 = mha_ln(cur, cur8, (w['sWq'], w['sWfc']), pre_s, g1, b1,
                         f"s{l}", pre_q=pq)
        if l + 1 < NL:
            # next layer's cross-kv hoists: K fills the s-attn scalar stalls,
            # V fills the c-attn scalar stalls (both only need att8)
            wstrk = tc.alloc_tile_pool(name=f"wstrk{l+1}", bufs=1)
            k_next = proj_stream(dt[f'cWk{l+1}'], att8, 2 * DC, f"kc{l+1}", "kTc", wstrk)
            wstrk.release()
            wstrv = tc.alloc_tile_pool(name=f"wstrv{l+1}", bufs=1)
            v_next = proj_v_stream(dt[f'cWv{l+1}'], att8, 2 * DC, f"c{l+1}", wstrv)
            wstrv.release()
            pre_c_next = (k_next, v_next)
        cur, cur8 = mha_ln(so, so8, (w['cWq'], w['cWfc']), pre_c, g2, b2, f"c{l}",
                     out_f32=(l == NL - 1), skip_ln=(l == NL - 1),
                     dma_row0=(2 * DC if l == NL - 1 else None))
        if l + 1 < NL:
            pre_c = pre_c_next
    wl1pool.release()
    ctx.close()


def _t128(a):
    # [n*128, w] -> [128, n*w] so each partition's DMA line is contiguous
    n = a.shape[0] // P
    return np.ascontiguousarray(
        a.reshape(n, P, a.shape[1]).transpose(1, 0, 2).reshape(P, -1))


def _t128pad(a):
    # ragged rows (LQ=160): pad to 2*128 rows then tile
    out = np.zeros((2 * P, a.shape[1]), a.dtype)
    out[:a.shape[0]] = a
    return _t128(out)


def kernel(**inputs):
    if 'nc' not in _CACHE:
        _CACHE['nc'] = _build()
    nc = _CACHE['nc']
    f = lambda x: np.ascontiguousarray(np.asarray(x), dtype=np.float32)
    bf = lambda x: np.asarray(x, dtype=np.float32).astype(NPBF)
    bfT = lambda x: np.asarray(x, dtype=np.float32).T.astype(NPBF)
    f8w = lambda x: np.clip(np.asarray(x, np.float32) * SW,
                            -240, 240).astype(NPE4)
    seq = f(inputs['sequences']); qry = f(inputs['query']); evd = f(inputs['evidence'])
    ke = f(inputs['knowledge_embed'])
    vecs = _t128(np.ascontiguousarray(np.stack(
        [f(inputs['w4C'])[:, 0], f(inputs['w4Q'])[:, 0],
         f(inputs['w4mlu'])[0, 0, :], f(inputs['cqa_b'])], axis=1)))
    cqa_WTf = np.ascontiguousarray(np.asarray(inputs['cqa_W'], np.float32).T)
    cqa_WTs = _t128(cqa_WTf[:D].astype(NPBF))
    cqa_WT8 = _t128(f8w(cqa_WTf[D:]))
    lwb = {}
    for l in range(NL):
        lwb[f'sWq{l}'] = _t128(f8w(inputs['L_sWq'][l]))
        lwb[f'sWk{l}'] = _t128(f8w(inputs['L_sWk'][l]))
        lwb[f'sWv{l}'] = _t128(f8w(inputs['L_sWv'][l]))
        lwb[f'sWfc{l}'] = _t128(f8w(inputs['L_sWfc'][l]))
        lwb[f'cWq{l}'] = _t128(f8w(inputs['L_cWq'][l]))
        lwb[f'cWk{l}'] = _t128(f8w(inputs['L_cWk'][l]))
        lwb[f'cWv{l}'] = _t128(f8w(inputs['L_cWv'][l]))
        lwb[f'cWfc{l}'] = _t128(f8w(inputs['L_cWfc'][l]))
        lwb[f'ln{l}'] = _t128(np.ascontiguousarray(np.stack(
            [f(inputs['L_n1g'][l]), f(inputs['L_n1b'][l]),
             f(inputs['L_n2g'][l]), f(inputs['L_n2b'][l])], axis=1)))
    in_maps = []
    for b in range(B):
        keT = np.ascontiguousarray(ke[b].T)
        m = {
            'S_nat': _t128(bf(seq[b])), 'S_T': _t128(bfT(seq[b])),
            'Q_nat': _t128pad(bf(qry[b])), 'Q_T': _t128(bfT(qry[b])),
            'E_nat': _t128pad(bf(evd[b])), 'E_T': _t128(bfT(evd[b])),
            'KE_T': _t128(keT.astype(NPBF)),
            'KE8': _t128(np.clip(keT, -240, 240).astype(NPE4)),
            'vecs': vecs, 'cqa_WTs': cqa_WTs, 'cqa_WT8': cqa_WT8,
        }
        m.update(lwb)
        in_maps.append(m)
    _CACHE['last_in_maps'] = in_maps
    res = run_bass_kernel_spmd(nc, in_maps, core_ids=list(range(B)))
    _CACHE['last_results'] = res
    outs = np.stack([np.asarray(r['out_t'], dtype=np.float32)
                     for r in res.results])                      # [B, 2304, 512]
    full = outs.transpose(0, 2, 1)                               # [B, 512, 2304]
    x1 = full[:, :, 2 * D:]                                      # pre-LN ke [B,512,768]
    muh = x1.mean(-1, keepdims=True)
    varh = x1.var(-1, keepdims=True)
    g = np.asarray(inputs['L_n2g'][NL - 1], dtype=np.float32)
    bb = np.asarray(inputs['L_n2b'][NL - 1], dtype=np.float32)
    ke_out = (x1 - muh) / np.sqrt(varh + EPS) * g + bb
    out = np.concatenate([seq, full[:, :, :2 * D], ke_out], axis=-1)
    return out

